# revision 1
# baseline (speedup 1.0000x reference)
"""GraphSAGE (2-layer, mean-agg) edge-scoring kernel for 8 trn2 NeuronCores.

  - Batch-parallel: core c handles edges [512c, 512(c+1)).
  - Projected tables sigmoid(feat @ W + b) in fp16 are built on-device into
    pair-shared HBM (cores 2k/2k+1 share one buffer; each projects half),
    synced with a pair AllReduce barrier.
  - Neighbor rows fetched with chunked dma_gather (int16 -> 25000-row
    chunks, <=1024 idx/call, 4 SWDGE queues), reordered/transposed to
    feat-major via SBUF-source transpose dma_gathers (hop-2 k-major so the
    10-way mean is 9 full-width vector adds; means folded into weights).
  - SAGE matmuls run feat-major (features on contraction partitions).
"""
import os
import numpy as np

F0 = F1 = 10
B = 4096
NCORES = 8
EDGES = B // NCORES          # 512
P = 128
D = 256
NU, NI = 50000, 100000
CHU = 25000                  # table chunk rows (int16-safe)
HALF_U, HALF_I = NU // 2, NI // 2
SEG_GROUPS = 640             # hop-2 groups per segment (= h1-token block)
NSEG = (EDGES * F0) // SEG_GROUPS    # 8
BLK = SEG_GROUPS
PROJ_TILE = 512


def _wrap16(a):
    a = np.asarray(a, np.int16)
    w = a.reshape(-1, 16).T
    return np.tile(w, (8, 1)).astype(np.int16)


def _pad128(n):
    return (n + 127) & ~127


class _HopPlan:
    def __init__(self, idx_lists, nrows, out_order, fixed_plen=None):
        self.nch = nrows // CHU
        M = len(idx_lists[0])
        runs = []
        for A in idx_lists:
            ch = A // CHU
            runs.append([np.where(ch == c)[0] for c in range(self.nch)])
        if fixed_plen is None:
            fixed_plen = [
                _pad128(max(len(r[c]) for r in runs)) for c in range(self.nch)]
        self.plen = fixed_plen
        self.runs = runs
        self.offs = np.concatenate([[0], np.cumsum(self.plen)]).astype(np.int64)
        self.tot = int(self.offs[-1])
        self.idx, self.rid = [], []
        for core, A in enumerate(idx_lists):
            iv = np.zeros(self.tot, np.int16)
            p2s = np.empty(M, np.int64)
            for c in range(self.nch):
                pos = runs[core][c]
                off = int(self.offs[c])
                iv[off:off + len(pos)] = (A[pos] - c * CHU).astype(np.int16)
                p2s[pos] = off + np.arange(len(pos))
            self.idx.append(iv)
            self.rid.append(p2s[out_order].astype(np.int16))
        self.calls = []
        for c in range(self.nch):
            off, rem = int(self.offs[c]), self.plen[c]
            while rem > 0:
                n = min(1024, rem)
                self.calls.append((c, off, n))
                off += n
                rem -= n


def _build_plans(inputs):
    plans = {}
    for side, (h0, h1, h2, t0, t1, t2) in {
        "s": (inputs["src_h0"], inputs["src_h1"], inputs["src_h2"], NU, NI, NU),
        "d": (inputs["dst_h0"], inputs["dst_h1"], inputs["dst_h2"], NI, NU, NI),
    }.items():
        h0 = np.asarray(h0).astype(np.int64).reshape(NCORES, EDGES)
        h1 = np.asarray(h1).astype(np.int64).reshape(NCORES, EDGES * F0)
        h2 = np.asarray(h2).astype(np.int64).reshape(NCORES, EDGES * F0 * F1)
        plans[side + "0"] = _HopPlan([h0[c] for c in range(NCORES)], t0,
                                     np.arange(EDGES))
        plans[side + "1"] = _HopPlan([h1[c] for c in range(NCORES)], t1,
                                     np.arange(EDGES * F0))
        # hop2 segments share one padded-run structure (max over cores+segs)
        oo = np.empty(SEG_GROUPS * F1, np.int64)
        for k in range(F1):
            oo[k * SEG_GROUPS:(k + 1) * SEG_GROUPS] = (
                np.arange(SEG_GROUPS) * F1 + k)
        nch = t2 // CHU
        seglists = [
            [h2[c][s * SEG_GROUPS * F1:(s + 1) * SEG_GROUPS * F1]
             for c in range(NCORES)] for s in range(NSEG)]
        plen = [0] * nch
        for s in range(NSEG):
            for A in seglists[s]:
                ch = A // CHU
                for c in range(nch):
                    plen[c] = max(plen[c], _pad128(int((ch == c).sum())))
        plans[side + "2"] = [
            _HopPlan(seglists[s], t2, oo, fixed_plen=plen) for s in range(NSEG)]
    return plans


def _proj_host(feat, half, ntiles):
    N = feat.shape[0]
    outs = []
    for parity in range(2):
        rows = np.arange(parity * half, (parity + 1) * half)
        padded = ntiles * PROJ_TILE
        rows_p = np.concatenate([rows, np.zeros(padded - half, np.int64)])
        order = rows_p.reshape(ntiles, P, 4).transpose(0, 2, 1).reshape(-1)
        # tile t, psum j, partition m -> original row order[t*512 + j*128 + m]
        xt = np.ascontiguousarray(feat[order].T.astype(np.float32))
        prow = np.empty((P, ntiles), np.int32)
        for t in range(ntiles):
            base = parity * half + t * PROJ_TILE
            pr = base + np.arange(P) * 4
            pr[pr >= (parity + 1) * half] = N
            prow[:, t] = pr // 4
        outs.append((xt, prow))
    return outs


def _build_bass(plans, ntu, nti, debug=False):
    import concourse.bass as bass
    import concourse.tile as tile
    import concourse.bacc as bacc
    from concourse import mybir, library_config
    from contextlib import ExitStack

    f16 = mybir.dt.float16
    f32 = mybir.dt.float32
    i16 = mybir.dt.int16
    i32 = mybir.dt.int32
    AF = mybir.ActivationFunctionType

    nc = bacc.Bacc("TRN2", target_bir_lowering=False, debug=False,
                   num_devices=NCORES, num_swdge_queues=4)

    xt_u = nc.dram_tensor("xt_u", [512, ntu * PROJ_TILE], f32, kind="ExternalInput")
    xt_i = nc.dram_tensor("xt_i", [512, nti * PROJ_TILE], f32, kind="ExternalInput")
    prow_u = nc.dram_tensor("prow_u", [P, ntu], i32, kind="ExternalInput")
    prow_i = nc.dram_tensor("prow_i", [P, nti], i32, kind="ExternalInput")
    w_pu = nc.dram_tensor("w_pu", [P, 4, D], f16, kind="ExternalInput")
    w_pi = nc.dram_tensor("w_pi", [P, 4, D], f16, kind="ExternalInput")
    b_p = nc.dram_tensor("b_p", [1, 2, D], f16, kind="ExternalInput")
    wsage = nc.dram_tensor("wsage", [P, 2, 2 * 768], f16, kind="ExternalInput")
    wlin = nc.dram_tensor("wlin", [P, 1], f16, kind="ExternalInput")
    blin = nc.dram_tensor("blin", [1, 1], f32, kind="ExternalInput")

    idx_t, rid_t = {}, {}
    for sd in ("s", "d"):
        p0, p1, seg2 = plans[sd + "0"], plans[sd + "1"], plans[sd + "2"]
        t2 = seg2[0].tot
        idx_t[sd + "0"] = nc.dram_tensor(f"idx{sd}0", [P, p0.tot // 16], i16,
                                         kind="ExternalInput")
        rid_t[sd + "0"] = nc.dram_tensor(f"rid{sd}0", [P, EDGES // 16], i16,
                                         kind="ExternalInput")
        idx_t[sd + "1"] = nc.dram_tensor(f"idx{sd}1", [P, p1.tot // 16], i16,
                                         kind="ExternalInput")
        rid_t[sd + "1"] = nc.dram_tensor(f"rid{sd}1", [P, EDGES * F0 // 16], i16,
                                         kind="ExternalInput")
        idx_t[sd + "2"] = nc.dram_tensor(f"idx{sd}2", [P, NSEG * t2 // 16], i16,
                                         kind="ExternalInput")
        rid_t[sd + "2"] = nc.dram_tensor(
            f"rid{sd}2", [P, NSEG * SEG_GROUPS * F1 // 16], i16,
            kind="ExternalInput")

    out = nc.dram_tensor("out", [1, EDGES], f32, kind="ExternalOutput")
    dbg = (nc.dram_tensor("dbg", [P, 2, EDGES], f32, kind="ExternalOutput")
           if debug else None)

    tab_u = nc.dram_tensor("tab_u", [NU + 4, D], f16, addr_space="Shared")
    tab_i = nc.dram_tensor("tab_i", [NI + 4, D], f16, addr_space="Shared")
    cc_in = nc.dram_tensor("cc_in", [1, 16], f32)
    cc_out = nc.dram_tensor("cc_out", [1, 16], f32)

    with tile.TileContext(nc) as tc, ExitStack() as ctx:
        nc.gpsimd.load_library(library_config.mlp)
        tc.strict_bb_all_engine_barrier()

        wpool = ctx.enter_context(tc.tile_pool(name="w", bufs=1))
        w_pu_s = wpool.tile([P, 4, D], f16, tag="wpu")
        w_pi_s = wpool.tile([P, 4, D], f16, tag="wpi")
        b_p_s = wpool.tile([1, 2, D], f16, tag="bp")
        wsage_s = wpool.tile([P, 2, 2 * 768], f16, tag="wsage")
        wlin_s = wpool.tile([P, 1], f16, tag="wlin")
        blin_s = wpool.tile([1, 1], f32, tag="blin")
        ones_s = wpool.tile([1, P], f16, tag="ones")
        for dst_, src_ in ((w_pu_s, w_pu), (w_pi_s, w_pi), (b_p_s, b_p),
                           (wsage_s, wsage), (wlin_s, wlin), (blin_s, blin)):
            nc.sync.dma_start(dst_[:], src_[:])
        nc.vector.memset(ones_s[:], 1.0)

        # ---------------- phase A: projection ----------------
        with tc.tile_pool(name="proj", bufs=3) as ppool, \
             tc.tile_pool(name="pps", bufs=2, space="PSUM") as pspool:

            def project(xt, prow, w_s, bcol, tab, ntiles):
                for t in range(ntiles):
                    xtt = ppool.tile([P, 4, PROJ_TILE], f16, tag="xtt")
                    nc.gpsimd.dma_start(
                        out=xtt[:],
                        in_=xt[:, t * PROJ_TILE:(t + 1) * PROJ_TILE].rearrange(
                            "(c p) n -> p c n", p=P))
                    prow_sb = ppool.tile([P, 1], i32, tag="prow")
                    nc.sync.dma_start(prow_sb[:], prow[:, t:t + 1])
                    sig = ppool.tile([P, 4, D], f16, tag="sig")
                    for j in range(4):
                        ps = pspool.tile([P, D], f32, tag="pps")
                        for c in range(4):
                            nc.tensor.matmul(
                                out=ps[:], lhsT=xtt[:, c, j * P:(j + 1) * P],
                                rhs=w_s[:, c, :], start=(c == 0), stop=False)
                        nc.tensor.matmul(out=ps[:], lhsT=ones_s[:, :],
                                         rhs=b_p_s[:, bcol, :], start=False,
                                         stop=True)
                        nc.scalar.activation(out=sig[:, j, :], in_=ps[:],
                                             func=AF.Sigmoid)
                    nc.gpsimd.indirect_dma_start(
                        out=tab[:, :].rearrange("(q r) d -> q (r d)", r=4),
                        out_offset=bass.IndirectOffsetOnAxis(
                            ap=prow_sb[:, :1], axis=0),
                        in_=sig[:].rearrange("p r d -> p (r d)"), in_offset=None)

            project(xt_u, prow_u, w_pu_s, 0, tab_u, ntu)
            project(xt_i, prow_i, w_pi_s, 1, tab_i, nti)

        # ---------------- phase B: pair barrier ----------------
        tc.strict_bb_all_engine_barrier()
        zz = wpool.tile([1, 16], f32, tag="zz")
        nc.vector.memset(zz[:], 1.0)
        nc.sync.dma_start(cc_in[:], zz[:])
        tc.strict_bb_all_engine_barrier()
        nc.gpsimd.collective_compute(
            "AllReduce", mybir.AluOpType.add,
            replica_groups=[[0, 1], [2, 3], [4, 5], [6, 7]],
            ins=[cc_in.ap()], outs=[cc_out.ap()])
        tc.strict_bb_all_engine_barrier()

        # ---------------- phase C: SAGE ----------------
        ipool = ctx.enter_context(tc.tile_pool(name="idx", bufs=1))
        i2pool = ctx.enter_context(tc.tile_pool(name="idx2", bufs=2))
        spool = ctx.enter_context(tc.tile_pool(name="stage", bufs=2))
        hpool = ctx.enter_context(tc.tile_pool(name="hts", bufs=1))
        kpool = ctx.enter_context(tc.tile_pool(name="kblk", bufs=2))
        vpool = ctx.enter_context(tc.tile_pool(name="vtmp", bufs=1))
        gpool = ctx.enter_context(tc.tile_pool(name="gts", bufs=1))
        ps2 = ctx.enter_context(tc.tile_pool(name="ps2", bufs=2, space="PSUM"))

        qn = [0]

        def gather_hbm(plan, idx_sb, coloff, tab, stage):
            for (c, off, n) in plan.calls:
                nc.gpsimd.dma_gather(
                    stage[:, off // P:(off + n) // P, :],
                    tab[c * CHU:(c + 1) * CHU, :],
                    idx_sb[:, coloff + off // 16: coloff + (off + n) // 16],
                    n, n, D, queue_num=qn[0] % 4)
                qn[0] += 1

        def regather(stage, rid_ap, n_out, dstT):
            nc.gpsimd.dma_gather(
                dstT[:], stage[:], rid_ap, n_out, n_out, D, transpose=True,
                sbuf_tokens_per_rank=P, sbuf_free_dim_per_rank=D * 2,
                queue_num=qn[0] % 4)
            qn[0] += 1

        def tree10_strided(src, dst, ngr):
            # src [P,2,ngr*10] fp16 (col j*10+k) -> dst [P,2,ngr] fp32
            t0_t = vpool.tile([P, 2, BLK], f32, tag="tr0")
            t0 = t0_t[:, :, :ngr]
            t1_t = vpool.tile([P, 2, BLK], f32, tag="tr1")
            t1 = t1_t[:, :, :ngr]
            v = src.rearrange("p c (j k) -> p c j k", k=F0)
            nc.vector.tensor_add(t0[:], v[:, :, :, 0], v[:, :, :, 1])
            for i in range(1, 5):
                nc.vector.tensor_add(t1[:], v[:, :, :, 2 * i], v[:, :, :, 2 * i + 1])
                if i < 4:
                    nc.vector.tensor_add(t0[:], t0[:], t1[:])
            nc.vector.tensor_add(dst, t0[:], t1[:])

        hts = {}
        for si, sd in enumerate(("s", "d")):
            p0, p1, seg2 = plans[sd + "0"], plans[sd + "1"], plans[sd + "2"]
            t2 = seg2[0].tot
            tA, tB = (tab_u, tab_i) if sd == "s" else (tab_i, tab_u)
            wof = si * 768
            ws0 = wsage_s[:, :, wof:wof + D]
            wa0 = wsage_s[:, :, wof + D:wof + 2 * D]
            ws1 = wsage_s[:, :, wof + 2 * D:wof + 2 * D + 128]
            wa1 = wsage_s[:, :, wof + 2 * D + 128:wof + 768]

            i0 = ipool.tile([P, p0.tot // 16], i16, tag="i0")
            nc.sync.dma_start(i0[:], idx_t[sd + "0"][:])
            r0 = ipool.tile([P, EDGES // 16], i16, tag="r0")
            nc.sync.dma_start(r0[:], rid_t[sd + "0"][:])
            i1 = ipool.tile([P, p1.tot // 16], i16, tag="i1")
            nc.sync.dma_start(i1[:], idx_t[sd + "1"][:])
            r1 = ipool.tile([P, EDGES * F0 // 16], i16, tag="r1")
            nc.sync.dma_start(r1[:], rid_t[sd + "1"][:])

            # --- h0 ---
            st0 = spool.tile([P, p0.tot // P, D], f16, tag="stg")
            gather_hbm(p0, i0, 0, tA, st0)
            h0T = hpool.tile([P, 2, EDGES], f16, tag="h0T")
            regather(st0, r0[:, :], EDGES, h0T)

            # --- h1 ---
            st1 = spool.tile([P, p1.tot // P, D], f16, tag="stg")
            gather_hbm(p1, i1, 0, tB, st1)
            h1T = hpool.tile([P, 2, EDGES * F0], f16, tag="h1T")
            n0f_t = vpool.tile([P, 2, EDGES], f32, tag="sumf")
            n0f = n0f_t[:, :, :EDGES]
            for b in range(EDGES * F0 // BLK):
                tmp = kpool.tile([P, 2, BLK], f16, tag="reT")
                regather(st1, r1[:, b * BLK // 16:(b + 1) * BLK // 16], BLK, tmp)
                nc.vector.tensor_copy(h1T[:, :, b * BLK:(b + 1) * BLK], tmp[:])
                tree10_strided(tmp[:], n0f[:, :, b * 64:(b + 1) * 64], 64)
            n0T = hpool.tile([P, 2, EDGES], f16, tag="n0T")
            nc.vector.tensor_copy(n0T[:], n0f[:])

            # --- h2 segments -> n1T ---
            n1T = hpool.tile([P, 2, EDGES * F0], f16, tag="n1T")
            for s in range(NSEG):
                pl = seg2[s]
                i2 = i2pool.tile([P, t2 // 16], i16, tag="i2")
                nc.sync.dma_start(i2[:], idx_t[sd + "2"][:, s * t2 // 16:
                                                         (s + 1) * t2 // 16])
                r2 = i2pool.tile([P, SEG_GROUPS * F1 // 16], i16, tag="r2")
                nc.sync.dma_start(
                    r2[:], rid_t[sd + "2"][:, s * SEG_GROUPS * F1 // 16:
                                           (s + 1) * SEG_GROUPS * F1 // 16])
                st2 = spool.tile([P, t2 // P, D], f16, tag="stg")
                gather_hbm(pl, i2, 0, tA, st2)
                t0 = vpool.tile([P, 2, BLK], f32, tag="tr0")
                t1 = vpool.tile([P, 2, BLK], f32, tag="tr1")
                ka = kpool.tile([P, 2, BLK], f16, tag="ka")
                kb = kpool.tile([P, 2, BLK], f16, tag="kb")
                for i in range(5):
                    regather(st2, r2[:, (2 * i) * BLK // 16:(2 * i + 1) * BLK // 16],
                             BLK, ka)
                    regather(st2, r2[:, (2 * i + 1) * BLK // 16:(2 * i + 2) * BLK // 16],
                             BLK, kb)
                    if i == 0:
                        nc.vector.tensor_add(t0[:], ka[:], kb[:])
                    else:
                        nc.vector.tensor_add(t1[:], ka[:], kb[:])
                        nc.vector.tensor_add(t0[:], t0[:], t1[:])
                nc.vector.tensor_copy(
                    n1T[:, :, s * SEG_GROUPS:(s + 1) * SEG_GROUPS], t0[:])

            # --- g1 = relu(h1 @ Ws0 + n1 @ Wa0) ---
            g1T = gpool.tile([P, 2, EDGES * F0], f16, tag="g1T")
            for o in range(2):
                for b in range(EDGES * F0 // BLK):
                    for half in range(2):
                        sl = slice(b * BLK + half * 320, b * BLK + (half + 1) * 320)
                        ps = ps2.tile([P, 320], f32, tag="g1ps")
                        for c in range(2):
                            nc.tensor.matmul(
                                out=ps[:], lhsT=ws0[:, c, o * P:(o + 1) * P],
                                rhs=h1T[:, c, sl], start=(c == 0), stop=False)
                            nc.tensor.matmul(
                                out=ps[:], lhsT=wa0[:, c, o * P:(o + 1) * P],
                                rhs=n1T[:, c, sl], start=False, stop=(c == 1))
                        nc.scalar.activation(out=g1T[:, o, sl], in_=ps[:],
                                             func=AF.Relu)

            nf_t = vpool.tile([P, 2, BLK], f32, tag="sumf")
            nf = nf_t[:, :, :EDGES]
            tree10_strided(g1T[:], nf, EDGES)
            nT = hpool.tile([P, 2, EDGES], f16, tag="nT")
            nc.vector.tensor_copy(nT[:], nf[:])

            # --- g0 = relu(h0 @ Ws0 + n0 @ Wa0) ---
            g0T = gpool.tile([P, 2, EDGES], f16, tag="g0T")
            for o in range(2):
                ps = ps2.tile([P, EDGES], f32, tag="mmps")
                for c in range(2):
                    nc.tensor.matmul(out=ps[:], lhsT=ws0[:, c, o * P:(o + 1) * P],
                                     rhs=h0T[:, c, :], start=(c == 0), stop=False)
                    nc.tensor.matmul(out=ps[:], lhsT=wa0[:, c, o * P:(o + 1) * P],
                                     rhs=n0T[:, c, :], start=False, stop=(c == 1))
                nc.scalar.activation(out=g0T[:, o, :], in_=ps[:], func=AF.Relu)

            # --- hT = g0 @ Ws1 + n @ Wa1 ---
            ps = ps2.tile([P, EDGES], f32, tag="mmps")
            for c in range(2):
                nc.tensor.matmul(out=ps[:], lhsT=ws1[:, c, :], rhs=g0T[:, c, :],
                                 start=(c == 0), stop=False)
                nc.tensor.matmul(out=ps[:], lhsT=wa1[:, c, :], rhs=nT[:, c, :],
                                 start=False, stop=(c == 1))
            hT = gpool.tile([P, EDGES], f16, tag=f"hT{sd}")
            nc.scalar.activation(out=hT[:], in_=ps[:], func=AF.Copy)
            hts[sd] = hT

        prod = gpool.tile([P, EDGES], f16, tag="prod")
        nc.vector.tensor_mul(prod[:], hts["s"][:], hts["d"][:])
        psf = ps2.tile([1, EDGES], f32, tag="fps")
        nc.tensor.matmul(out=psf[:], lhsT=wlin_s[:], rhs=prod[:],
                         start=True, stop=True)
        res = gpool.tile([1, EDGES], f32, tag="res")
        nc.scalar.activation(out=res[:], in_=psf[:], func=AF.Identity,
                             bias=blin_s[:, :1])
        nc.sync.dma_start(out[:], res[:])
        if debug:
            dv = gpool.tile([P, 2, EDGES], f32, tag="dv")
            nc.vector.tensor_copy(dv[:, 0, :], hts["s"][:])
            nc.vector.tensor_copy(dv[:, 1, :], hts["d"][:])
            nc.sync.dma_start(dbg[:], dv[:])

    nc.compile()
    return nc


def kernel(**inputs) -> np.ndarray:
    from concourse.bass_utils import run_bass_kernel_spmd

    plans = _build_plans(inputs)
    ntu = -(-HALF_U // PROJ_TILE)   # 49
    nti = -(-HALF_I // PROJ_TILE)   # 98

    trace = bool(os.environ.get("GNN_TRACE"))
    debug = bool(os.environ.get("GNN_DEBUG"))
    if trace:
        import timing_shim
        timing_shim.install()

    nc = _build_bass(plans, ntu, nti, debug=debug)

    uf = np.asarray(inputs["user_feat"], np.float32)
    itf = np.asarray(inputs["item_feat"], np.float32)
    proj_u = _proj_host(uf, HALF_U, ntu)
    proj_i = _proj_host(itf, HALF_I, nti)

    f16 = np.float16
    w_pu = np.ascontiguousarray(
        np.asarray(inputs["W_pu"], np.float32).reshape(4, P, D)
        .transpose(1, 0, 2)).astype(f16)
    w_pi = np.ascontiguousarray(
        np.asarray(inputs["W_pi"], np.float32).reshape(4, P, D)
        .transpose(1, 0, 2)).astype(f16)
    b_p = np.stack([np.asarray(inputs["b_pu"], np.float32),
                    np.asarray(inputs["b_pi"], np.float32)])[None].astype(f16)

    def sagew(pre):
        s0 = np.asarray(inputs[f"{pre}_self0"], np.float32)
        a0 = np.asarray(inputs[f"{pre}_agg0"], np.float32) * (1.0 / F0)
        s1 = np.asarray(inputs[f"{pre}_self1"], np.float32)
        a1 = np.asarray(inputs[f"{pre}_agg1"], np.float32) * (1.0 / F0)
        cat = np.concatenate([s0, a0, s1, a1], axis=1)  # [256, 768]
        return cat.reshape(2, P, 768).transpose(1, 0, 2)

    wsage = np.ascontiguousarray(
        np.concatenate([sagew("u"), sagew("i")], axis=2)).astype(f16)
    wlin = np.asarray(inputs["W_lin"], np.float32).astype(f16)
    blin = np.asarray(inputs["b_lin"], np.float32).reshape(1, 1)

    in_maps = []
    for c in range(NCORES):
        par = c % 2
        m = {
            "xt_u": proj_u[par][0], "prow_u": proj_u[par][1],
            "xt_i": proj_i[par][0], "prow_i": proj_i[par][1],
            "w_pu": w_pu, "w_pi": w_pi, "b_p": b_p,
            "wsage": wsage, "wlin": wlin, "blin": blin,
        }
        for sd in ("s", "d"):
            p0, p1, seg2 = plans[sd + "0"], plans[sd + "1"], plans[sd + "2"]
            m[f"idx{sd}0"] = _wrap16(p0.idx[c])
            m[f"rid{sd}0"] = _wrap16(p0.rid[c])
            m[f"idx{sd}1"] = _wrap16(p1.idx[c])
            m[f"rid{sd}1"] = _wrap16(p1.rid[c])
            m[f"idx{sd}2"] = np.concatenate(
                [_wrap16(pl.idx[c]) for pl in seg2], axis=1)
            m[f"rid{sd}2"] = np.concatenate(
                [_wrap16(pl.rid[c]) for pl in seg2], axis=1)
        in_maps.append(m)

    kw = dict(trace=True, trace_cores=list(range(NCORES))) if trace else {}
    res = run_bass_kernel_spmd(nc, in_maps, core_ids=list(range(NCORES)), **kw)
    if trace and res.exec_time_ns:
        print(f"HW exec time: {res.exec_time_ns} ns")
        kernel.last_exec_ns = res.exec_time_ns
    if debug:
        kernel.last_dbg = [res.results[c]["dbg"] for c in range(NCORES)]

    logits = np.concatenate([res.results[c]["out"][0] for c in range(NCORES)])
    return logits.reshape(B, 1).astype(np.float32)



# revision 16
# speedup vs baseline: 1.7336x; 1.7336x over previous
"""GraphSAGE (2-layer, mean-agg) edge-scoring kernel for 8 trn2 NeuronCores.

  - Batch-parallel: core c handles edges [512c, 512(c+1)).
  - Projected tables sigmoid(feat @ W + b) in fp16 are built on-device into
    pair-shared HBM (cores 2k/2k+1 share one buffer; each projects half),
    synced with a pair AllReduce barrier. Features shipped as fp16 so the
    loads run on the sync HWDGE engine (no cast) at half the bytes.
  - Neighbor rows fetched with chunked dma_gather (int16 -> 25000-row
    chunks, <=1024 idx/call, 4 SWDGE queues).
  - Hop means computed on the Tensor engine: per 128-slot stage tile,
    matmul(lhsT=stage_tile, rhs=G_tile) accumulates sum-of-group-rows
    directly in PSUM, feat-major (G = host-built banded 0/1 selection,
    streamed from HBM). Kills the SBUF transpose-regather pass for hop-2
    and the vector tree-adds (means folded into weights).
  - SAGE matmuls run feat-major (features on contraction partitions).
"""
import os
import numpy as np

F0 = F1 = 10
B = 4096
NCORES = 8
EDGES = B // NCORES          # 512
P = 128
D = 256
NU, NI = 50000, 100000
CHU = 25000                  # table chunk rows (int16-safe)
HALF_U, HALF_I = NU // 2, NI // 2
SEG_GROUPS = 512             # hop-2 groups per segment (one PSUM bank wide)
NSEG = (EDGES * F0) // SEG_GROUPS    # 10
BLK = 640                    # h1 regather block
PROJ_TILE = 512
RPP = 4                      # table rows packed per partition in scatter


def _wrap16(a):
    a = np.asarray(a, np.int16)
    w = a.reshape(-1, 16).T
    return np.tile(w, (8, 1)).astype(np.int16)


def _pad128(n):
    return (n + 127) & ~127


class _HopPlan:
    def __init__(self, idx_lists, nrows, out_order, fixed_plen=None):
        self.nch = nrows // CHU
        M = len(idx_lists[0])
        runs = []
        for A in idx_lists:
            ch = A // CHU
            runs.append([np.where(ch == c)[0] for c in range(self.nch)])
        if fixed_plen is None:
            fixed_plen = [
                _pad128(max(len(r[c]) for r in runs)) for c in range(self.nch)]
        self.plen = fixed_plen
        self.runs = runs
        self.offs = np.concatenate([[0], np.cumsum(self.plen)]).astype(np.int64)
        self.tot = int(self.offs[-1])
        self.idx, self.rid = [], []
        for core, A in enumerate(idx_lists):
            iv = np.zeros(self.tot, np.int16)
            p2s = np.empty(M, np.int64)
            for c in range(self.nch):
                pos = runs[core][c]
                off = int(self.offs[c])
                iv[off:off + len(pos)] = (A[pos] - c * CHU).astype(np.int16)
                p2s[pos] = off + np.arange(len(pos))
            self.idx.append(iv)
            self.rid.append(p2s[out_order].astype(np.int16))
        self.calls = []
        for c in range(self.nch):
            off, rem = int(self.offs[c]), self.plen[c]
            while rem > 0:
                n = min(1024, rem)
                self.calls.append((c, off, n))
                off += n
                rem -= n


def _build_G(plans, ngroups, fanout):
    """Banded group-sum selection matrices for a list of per-seg plans.
    Returns (wmax, c0[nseg,ntiles], nonempty[nseg,ntiles],
    G[NCORES, nseg, ntiles, P, wmax] fp16). c0/nonempty are core-uniform
    (baked into the shared program); G is per-core input data."""
    nseg = len(plans)
    ntiles = plans[0].tot // P
    gmin = np.full((nseg, ntiles), 10**9, np.int64)
    gmax = np.full((nseg, ntiles), -1, np.int64)
    for s, pl in enumerate(plans):
        for core in range(NCORES):
            for c in range(pl.nch):
                pos = pl.runs[core][c]
                if len(pos) == 0:
                    continue
                slots = pl.offs[c] + np.arange(len(pos))
                grp = pos // fanout
                t = slots // P
                np.minimum.at(gmin, (s, t), grp)
                np.maximum.at(gmax, (s, t), grp)
    nonempty = gmax >= 0
    wmax = int((gmax[nonempty] - gmin[nonempty] + 1).max())
    wmax = (wmax + 7) & ~7
    assert wmax <= ngroups
    c0 = np.minimum(np.where(nonempty, gmin, 0), ngroups - wmax)
    c0 = np.maximum(c0, 0).astype(np.int64)
    G = np.zeros((NCORES, nseg, ntiles, P, wmax), np.float16)
    for s, pl in enumerate(plans):
        for core in range(NCORES):
            for c in range(pl.nch):
                pos = pl.runs[core][c]
                if len(pos) == 0:
                    continue
                slots = pl.offs[c] + np.arange(len(pos))
                grp = pos // fanout
                t = slots // P
                p = slots % P
                col = grp - c0[s, t]
                G[core, s, t, p, col] = 1.0
    return wmax, c0, nonempty, G


def _build_plans(inputs):
    plans = {}
    for side, (h0, h1, h2, t0, t1, t2) in {
        "s": (inputs["src_h0"], inputs["src_h1"], inputs["src_h2"], NU, NI, NU),
        "d": (inputs["dst_h0"], inputs["dst_h1"], inputs["dst_h2"], NI, NU, NI),
    }.items():
        h0 = np.asarray(h0).astype(np.int64).reshape(NCORES, EDGES)
        h1 = np.asarray(h1).astype(np.int64).reshape(NCORES, EDGES * F0)
        h2 = np.asarray(h2).astype(np.int64).reshape(NCORES, EDGES * F0 * F1)
        plans[side + "0"] = _HopPlan([h0[c] for c in range(NCORES)], t0,
                                     np.arange(EDGES))
        p1 = _HopPlan([h1[c] for c in range(NCORES)], t1,
                      np.arange(EDGES * F0))
        plans[side + "1"] = p1
        plans[side + "1G"] = _build_G([p1], EDGES, F0)
        # hop2 segments share one padded-run structure (max over cores+segs)
        nch = t2 // CHU
        seglists = [
            [h2[c][s * SEG_GROUPS * F1:(s + 1) * SEG_GROUPS * F1]
             for c in range(NCORES)] for s in range(NSEG)]
        plen = [0] * nch
        for s in range(NSEG):
            for A in seglists[s]:
                ch = A // CHU
                for c in range(nch):
                    plen[c] = max(plen[c], _pad128(int((ch == c).sum())))
        seg2 = [_HopPlan(seglists[s], t2, np.arange(SEG_GROUPS * F1),
                         fixed_plen=plen) for s in range(NSEG)]
        plans[side + "2"] = seg2
        plans[side + "2G"] = _build_G(seg2, SEG_GROUPS, F1)
    return plans


def _proj_host(feat, half, ntiles):
    N = feat.shape[0]
    outs = []
    for parity in range(2):
        rows = np.arange(parity * half, (parity + 1) * half)
        padded = ntiles * PROJ_TILE
        rows_p = np.concatenate([rows, np.zeros(padded - half, np.int64)])
        order = rows_p.reshape(ntiles, P, RPP).transpose(0, 2, 1).reshape(-1)
        # tile t, psum j, partition m -> original row order[t*T + j*128 + m]
        xt = feat[order].T.astype(np.float16)          # [512 feats, ntiles*T]
        # partition-major: xt2[p, t*4T + c*T + n] = xt[c*128+p, t*T+n]
        xt = np.ascontiguousarray(
            xt.reshape(4, P, ntiles, PROJ_TILE).transpose(1, 2, 0, 3)
            .reshape(P, ntiles * 4 * PROJ_TILE))
        prow = np.empty((P, ntiles), np.int32)
        for t in range(ntiles):
            base = parity * half + t * PROJ_TILE
            pr = base + np.arange(P) * RPP
            pr[pr >= (parity + 1) * half] = N
            prow[:, t] = pr // RPP
        outs.append((xt, prow))
    return outs


def _build_bass(plans, ntu, nti, debug=False):
    import concourse.bass as bass
    import concourse.tile as tile
    import concourse.bacc as bacc
    from concourse import mybir, library_config
    from contextlib import ExitStack

    f16 = mybir.dt.float16
    f32 = mybir.dt.float32
    i16 = mybir.dt.int16
    i32 = mybir.dt.int32
    AF = mybir.ActivationFunctionType

    nc = bacc.Bacc("TRN2", target_bir_lowering=False, debug=False,
                   num_devices=NCORES, num_swdge_queues=4)

    xt_u = nc.dram_tensor("xt_u", [P, ntu * 4 * PROJ_TILE], f16,
                          kind="ExternalInput")
    xt_i = nc.dram_tensor("xt_i", [P, nti * 4 * PROJ_TILE], f16,
                          kind="ExternalInput")
    prow_u = nc.dram_tensor("prow_u", [P, ntu], i32, kind="ExternalInput")
    prow_i = nc.dram_tensor("prow_i", [P, nti], i32, kind="ExternalInput")
    w_pu = nc.dram_tensor("w_pu", [P, 4, D], f16, kind="ExternalInput")
    w_pi = nc.dram_tensor("w_pi", [P, 4, D], f16, kind="ExternalInput")
    b_bc = nc.dram_tensor("b_bc", [P, 2, D], f32, kind="ExternalInput")
    wsage = nc.dram_tensor("wsage", [P, 2, 2 * 768], f16, kind="ExternalInput")
    wlin = nc.dram_tensor("wlin", [P, 1], f16, kind="ExternalInput")
    blin = nc.dram_tensor("blin", [1, 1], f32, kind="ExternalInput")

    idx_t, rid_t, g_t = {}, {}, {}
    for sd in ("s", "d"):
        p0, p1, seg2 = plans[sd + "0"], plans[sd + "1"], plans[sd + "2"]
        w1 = plans[sd + "1G"][0]
        w2 = plans[sd + "2G"][0]
        nt1 = p1.tot // P
        nt2 = seg2[0].tot // P
        t2 = seg2[0].tot
        idx_t[sd + "0"] = nc.dram_tensor(f"idx{sd}0", [P, p0.tot // 16], i16,
                                         kind="ExternalInput")
        rid_t[sd + "0"] = nc.dram_tensor(f"rid{sd}0", [P, EDGES // 16], i16,
                                         kind="ExternalInput")
        idx_t[sd + "1"] = nc.dram_tensor(f"idx{sd}1", [P, p1.tot // 16], i16,
                                         kind="ExternalInput")
        rid_t[sd + "1"] = nc.dram_tensor(f"rid{sd}1", [P, EDGES * F0 // 16], i16,
                                         kind="ExternalInput")
        idx_t[sd + "2"] = nc.dram_tensor(f"idx{sd}2", [P, NSEG * t2 // 16], i16,
                                         kind="ExternalInput")
        g_t[sd + "1"] = nc.dram_tensor(f"g{sd}1", [P, nt1 * w1], f16,
                                       kind="ExternalInput")
        g_t[sd + "2"] = nc.dram_tensor(f"g{sd}2", [P, NSEG * nt2 * w2], f16,
                                       kind="ExternalInput")

    out = nc.dram_tensor("out", [1, EDGES], f32, kind="ExternalOutput")
    dbg = (nc.dram_tensor("dbg", [P, 2, EDGES], f32, kind="ExternalOutput")
           if debug else None)

    tab_u = nc.dram_tensor("tab_u", [NU + RPP, D], f16, addr_space="Shared")
    tab_i = nc.dram_tensor("tab_i", [NI + RPP, D], f16, addr_space="Shared")
    cc_in1 = nc.dram_tensor("cc_in1", [1, 16], f32)
    cc_out1 = nc.dram_tensor("cc_out1", [1, 16], f32)
    cc_in2 = nc.dram_tensor("cc_in2", [1, 16], f32)
    cc_out2 = nc.dram_tensor("cc_out2", [1, 16], f32)

    with tile.TileContext(nc) as tc, ExitStack() as ctx:
        nc.gpsimd.load_library(library_config.mlp)
        tc.strict_bb_all_engine_barrier()

        wpool = ctx.enter_context(tc.tile_pool(name="w", bufs=1))
        w_pu_s = wpool.tile([P, 4, D], f16, tag="wpu")
        w_pi_s = wpool.tile([P, 4, D], f16, tag="wpi")
        b_bc_s = wpool.tile([P, 2, D], f32, tag="bbc")
        wsage_s = wpool.tile([P, 2, 2 * 768], f16, tag="wsage")
        wlin_s = wpool.tile([P, 1], f16, tag="wlin")
        blin_s = wpool.tile([1, 1], f32, tag="blin")
        for dst_, src_ in ((w_pu_s, w_pu), (w_pi_s, w_pi), (b_bc_s, b_bc),
                           (wsage_s, wsage), (wlin_s, wlin), (blin_s, blin)):
            nc.sync.dma_start(dst_[:], src_[:])

        # ---------------- phase A: projection ----------------
        ppool = ctx.enter_context(tc.tile_pool(name="proj", bufs=2))
        pspool = ctx.enter_context(tc.tile_pool(name="pps", bufs=2,
                                                space="PSUM"))

        # PE warmup: ~16 back-to-back matmuls (~4.4us cold) so the HAM
        # clock gate opens to 2.4 GHz before the projection stream.
        wu = wpool.tile([P, 512], f16, tag="warm")
        nc.vector.memset(wu[:], 0.0)
        psw = pspool.tile([P, D], f32, tag="pps")
        for i in range(16):
            nc.tensor.matmul(out=psw[:], lhsT=wu[:, :P], rhs=wu[:, 256:],
                             start=(i == 0), stop=(i == 15))
        zz = wpool.tile([1, 16], f32, tag="zz")
        nc.vector.memset(zz[:], 1.0)

        def project(xt, prow, w_s, bcol, tab, ntiles):
            for t in range(ntiles):
                xtt = ppool.tile([P, 4, PROJ_TILE], f16, tag="xtt")
                nc.sync.dma_start(
                    out=xtt[:],
                    in_=xt[:, t * 4 * PROJ_TILE:(t + 1) * 4 * PROJ_TILE]
                    .rearrange("p (c n) -> p c n", c=4))
                prow_sb = ppool.tile([P, 1], i32, tag="prow")
                nc.sync.dma_start(prow_sb[:], prow[:, t:t + 1])
                sig = ppool.tile([P, RPP, D], f16, tag="sig")
                for j in range(RPP):
                    ps = pspool.tile([P, D], f32, tag="pps")
                    nc.vector.tensor_copy(ps[:], b_bc_s[:, bcol, :])
                    for c in range(4):
                        nc.tensor.matmul(
                            out=ps[:], lhsT=xtt[:, c, j * P:(j + 1) * P],
                            rhs=w_s[:, c, :], start=False, stop=(c == 3))
                    nc.scalar.activation(out=sig[:, j, :], in_=ps[:],
                                         func=AF.Sigmoid)
                nc.gpsimd.indirect_dma_start(
                    out=tab[:, :].rearrange("(q r) d -> q (r d)", r=RPP),
                    out_offset=bass.IndirectOffsetOnAxis(
                        ap=prow_sb[:, :1], axis=0),
                    in_=sig[:].rearrange("p r d -> p (r d)"), in_offset=None)

        def pair_sync(cc_in, cc_out):
            tc.strict_bb_all_engine_barrier()
            nc.sync.dma_start(cc_in[:], zz[:])
            tc.strict_bb_all_engine_barrier()
            nc.gpsimd.collective_compute(
                "AllReduce", mybir.AluOpType.add,
                replica_groups=[[0, 1], [2, 3], [4, 5], [6, 7]],
                ins=[cc_in.ap()], outs=[cc_out.ap()])
            tc.strict_bb_all_engine_barrier()

        # ---------------- phase C pools ----------------
        ipool = ctx.enter_context(tc.tile_pool(name="idx", bufs=1))
        i2pool = ctx.enter_context(tc.tile_pool(name="idx2", bufs=2))
        gpool2 = ctx.enter_context(tc.tile_pool(name="gmat", bufs=1))
        spool = ctx.enter_context(tc.tile_pool(name="stage", bufs=2))
        s1pool = ctx.enter_context(tc.tile_pool(name="stage1", bufs=1))
        hpool = ctx.enter_context(tc.tile_pool(name="hts", bufs=1))
        kpool = ctx.enter_context(tc.tile_pool(name="kblk", bufs=2))
        vpool = ctx.enter_context(tc.tile_pool(name="vtmp", bufs=1))
        gpool = ctx.enter_context(tc.tile_pool(name="gts", bufs=1))
        ps2 = ctx.enter_context(tc.tile_pool(name="ps2", bufs=2, space="PSUM"))
        aggps = ctx.enter_context(tc.tile_pool(name="aggps", bufs=1,
                                               space="PSUM"))

        qn = [0]

        def gather_hbm(plan, idx_sb, tab, stage):
            for (c, off, n) in plan.calls:
                nc.gpsimd.dma_gather(
                    stage[:, off // P:(off + n) // P, :],
                    tab[c * CHU:(c + 1) * CHU, :],
                    idx_sb[:, off // 16:(off + n) // 16],
                    n, n, D, queue_num=qn[0] % 4)
                qn[0] += 1

        def regather(stage, rid_ap, n_out, dstT):
            nc.gpsimd.dma_gather(
                dstT[:], stage[:], rid_ap, n_out, n_out, D, transpose=True,
                sbuf_tokens_per_rank=P, sbuf_free_dim_per_rank=D * 2,
                queue_num=qn[0] % 4)
            qn[0] += 1

        def agg_matmul(stage, gt, wmax, c0, nonempty, s, pslo, pshi):
            ntiles = stage.shape[1]
            nc.vector.memset(pslo[:], 0.0)
            nc.vector.memset(pshi[:], 0.0)
            for t in range(ntiles):
                if not nonempty[s, t]:
                    continue
                a = int(c0[s, t])
                rhs = gt[:, t * wmax:(t + 1) * wmax]
                nc.tensor.matmul(out=pslo[:, a:a + wmax],
                                 lhsT=stage[:, t, 0:P], rhs=rhs,
                                 start=False, stop=False)
                nc.tensor.matmul(out=pshi[:, a:a + wmax],
                                 lhsT=stage[:, t, P:D], rhs=rhs,
                                 start=False, stop=False)

        def tree10_strided(src, dst, ngr):
            # src [P,2,ngr*10] fp16 (col j*10+k) -> dst [P,2,ngr] f16
            t0_t = vpool.tile([P, 2, EDGES], f16, tag="tr0")
            t0 = t0_t[:, :, :ngr]
            t1_t = vpool.tile([P, 2, EDGES], f16, tag="tr1")
            t1 = t1_t[:, :, :ngr]
            v = src.rearrange("p c (j k) -> p c j k", k=F0)
            nc.vector.tensor_add(t0[:], v[:, :, :, 0], v[:, :, :, 1])
            for i in range(1, 5):
                nc.vector.tensor_add(t1[:], v[:, :, :, 2 * i], v[:, :, :, 2 * i + 1])
                if i < 4:
                    nc.vector.tensor_add(t0[:], t0[:], t1[:])
            nc.vector.tensor_add(dst, t0[:], t1[:])

        def side_w(si):
            wof = si * 768
            return (wsage_s[:, :, wof:wof + D],
                    wsage_s[:, :, wof + D:wof + 2 * D],
                    wsage_s[:, :, wof + 2 * D:wof + 2 * D + 128],
                    wsage_s[:, :, wof + 2 * D + 128:wof + 768])

        S = {"s": {}, "d": {}}

        def do_h0(sd, tab):
            p0 = plans[sd + "0"]
            i0 = ipool.tile([P, p0.tot // 16], i16, tag=f"i0{sd}")
            nc.sync.dma_start(i0[:], idx_t[sd + "0"][:])
            r0 = ipool.tile([P, EDGES // 16], i16, tag=f"r0{sd}")
            nc.sync.dma_start(r0[:], rid_t[sd + "0"][:])
            st0 = spool.tile([P, p0.tot // P, D], f16, tag="stg")
            gather_hbm(p0, i0, tab, st0)
            h0T = hpool.tile([P, 2, EDGES], f16, tag=f"h0T{sd}")
            regather(st0, r0[:, :], EDGES, h0T)
            S[sd]["h0T"] = h0T

        def do_h2(sd, tab):
            seg2 = plans[sd + "2"]
            w2, c02, ne2, _ = plans[sd + "2G"]
            t2 = seg2[0].tot
            nt2 = t2 // P
            n1T = hpool.tile([P, 2, EDGES * F0], f16, tag="n1T")
            for s in range(NSEG):
                pl = seg2[s]
                i2 = i2pool.tile([P, t2 // 16], i16, tag="i2")
                nc.sync.dma_start(i2[:], idx_t[sd + "2"][:, s * t2 // 16:
                                                         (s + 1) * t2 // 16])
                gt = gpool2.tile([P, nt2 * w2], f16, tag="gt")
                nc.sync.dma_start(
                    gt[:], g_t[sd + "2"][:, s * nt2 * w2:(s + 1) * nt2 * w2])
                st2 = spool.tile([P, nt2, D], f16, tag="stg")
                gather_hbm(pl, i2, tab, st2)
                pslo = aggps.tile([P, SEG_GROUPS], f32, tag="agglo")
                pshi = aggps.tile([P, SEG_GROUPS], f32, tag="agghi")
                agg_matmul(st2, gt, w2, c02, ne2, s, pslo, pshi)
                sl = slice(s * SEG_GROUPS, (s + 1) * SEG_GROUPS)
                nc.scalar.activation(out=n1T[:, 0, sl], in_=pslo[:], func=AF.Copy)
                nc.scalar.activation(out=n1T[:, 1, sl], in_=pshi[:], func=AF.Copy)
            S[sd]["n1T"] = n1T

        def do_h1(sd, tab):
            p1 = plans[sd + "1"]
            w1, c01, ne1, _ = plans[sd + "1G"]
            nt1 = p1.tot // P
            i1 = ipool.tile([P, p1.tot // 16], i16, tag=f"i1{sd}")
            nc.sync.dma_start(i1[:], idx_t[sd + "1"][:])
            r1 = ipool.tile([P, EDGES * F0 // 16], i16, tag=f"r1{sd}")
            nc.sync.dma_start(r1[:], rid_t[sd + "1"][:])
            g1sb = ipool.tile([P, nt1 * w1], f16, tag="g1sb")
            nc.sync.dma_start(g1sb[:], g_t[sd + "1"][:])
            st1 = s1pool.tile([P, nt1, D], f16, tag="stg1")
            gather_hbm(p1, i1, tab, st1)
            h1T = hpool.tile([P, 2, EDGES * F0], f16, tag="h1T")
            for b in range(EDGES * F0 // BLK):
                tmp = kpool.tile([P, 2, BLK], f16, tag="reT")
                regather(st1, r1[:, b * BLK // 16:(b + 1) * BLK // 16], BLK, tmp)
                nc.vector.tensor_copy(h1T[:, :, b * BLK:(b + 1) * BLK], tmp[:])
            pslo = aggps.tile([P, EDGES], f32, tag="agglo")
            pshi = aggps.tile([P, EDGES], f32, tag="agghi")
            agg_matmul(st1, g1sb, w1, c01, ne1, 0, pslo, pshi)
            n0T = hpool.tile([P, 2, EDGES], f16, tag=f"n0T{sd}")
            nc.scalar.activation(out=n0T[:, 0, :], in_=pslo[:], func=AF.Copy)
            nc.scalar.activation(out=n0T[:, 1, :], in_=pshi[:], func=AF.Copy)
            S[sd]["h1T"] = h1T
            S[sd]["n0T"] = n0T

        hts = {}

        def do_sage(sd, si):
            ws0, wa0, ws1, wa1 = side_w(si)
            h0T, h1T = S[sd]["h0T"], S[sd]["h1T"]
            n0T, n1T = S[sd]["n0T"], S[sd]["n1T"]
            g1T = gpool.tile([P, 2, EDGES * F0], f16, tag="g1T")
            for o in range(2):
                for b in range(EDGES * F0 // BLK):
                    for half in range(2):
                        sl = slice(b * BLK + half * 320, b * BLK + (half + 1) * 320)
                        ps = ps2.tile([P, 320], f32, tag="g1ps")
                        for c in range(2):
                            nc.tensor.matmul(
                                out=ps[:], lhsT=ws0[:, c, o * P:(o + 1) * P],
                                rhs=h1T[:, c, sl], start=(c == 0), stop=False)
                            nc.tensor.matmul(
                                out=ps[:], lhsT=wa0[:, c, o * P:(o + 1) * P],
                                rhs=n1T[:, c, sl], start=False, stop=(c == 1))
                        nc.scalar.activation(out=g1T[:, o, sl], in_=ps[:],
                                             func=AF.Relu)

            nT = hpool.tile([P, 2, EDGES], f16, tag=f"nT{sd}")
            tree10_strided(g1T[:], nT[:], EDGES)

            g0T = gpool.tile([P, 2, EDGES], f16, tag="g0T")
            for o in range(2):
                ps = ps2.tile([P, EDGES], f32, tag="mmps")
                for c in range(2):
                    nc.tensor.matmul(out=ps[:], lhsT=ws0[:, c, o * P:(o + 1) * P],
                                     rhs=h0T[:, c, :], start=(c == 0), stop=False)
                    nc.tensor.matmul(out=ps[:], lhsT=wa0[:, c, o * P:(o + 1) * P],
                                     rhs=n0T[:, c, :], start=False, stop=(c == 1))
                nc.scalar.activation(out=g0T[:, o, :], in_=ps[:], func=AF.Relu)

            ps = ps2.tile([P, EDGES], f32, tag="mmps")
            for c in range(2):
                nc.tensor.matmul(out=ps[:], lhsT=ws1[:, c, :], rhs=g0T[:, c, :],
                                 start=(c == 0), stop=False)
                nc.tensor.matmul(out=ps[:], lhsT=wa1[:, c, :], rhs=nT[:, c, :],
                                 start=False, stop=(c == 1))
            hT = gpool.tile([P, EDGES], f16, tag=f"hT{sd}")
            nc.scalar.activation(out=hT[:], in_=ps[:], func=AF.Copy)
            hts[sd] = hT

        # ---------------- pipeline ----------------
        project(xt_u, prow_u, w_pu_s, 0, tab_u, ntu)
        pair_sync(cc_in1, cc_out1)
        # item-table projection overlaps all tab_u-side work below
        project(xt_i, prow_i, w_pi_s, 1, tab_i, nti)
        do_h0("s", tab_u)
        do_h2("s", tab_u)
        pair_sync(cc_in2, cc_out2)
        do_h1("s", tab_i)
        do_sage("s", 0)
        do_h1("d", tab_u)
        do_h0("d", tab_i)
        do_h2("d", tab_i)
        do_sage("d", 1)

        prod = gpool.tile([P, EDGES], f16, tag="prod")
        nc.vector.tensor_mul(prod[:], hts["s"][:], hts["d"][:])
        psf = ps2.tile([1, EDGES], f32, tag="mmps")
        nc.tensor.matmul(out=psf[:], lhsT=wlin_s[:], rhs=prod[:],
                         start=True, stop=True)
        res = gpool.tile([1, EDGES], f32, tag="res")
        nc.scalar.activation(out=res[:], in_=psf[:], func=AF.Identity,
                             bias=blin_s[:, :1])
        nc.sync.dma_start(out[:], res[:])
        if debug:
            dv = gpool.tile([P, 2, EDGES], f32, tag="dv")
            nc.vector.tensor_copy(dv[:, 0, :], hts["s"][:])
            nc.vector.tensor_copy(dv[:, 1, :], hts["d"][:])
            nc.sync.dma_start(dbg[:], dv[:])

    nc.compile()
    return nc


def kernel(**inputs) -> np.ndarray:
    plans = _build_plans(inputs)
    ntu = -(-HALF_U // PROJ_TILE)   # 49
    nti = -(-HALF_I // PROJ_TILE)   # 98

    trace = bool(os.environ.get("GNN_TRACE"))
    debug = bool(os.environ.get("GNN_DEBUG"))
    if trace:
        import timing_shim
        timing_shim.install()
    from concourse.bass_utils import run_bass_kernel_spmd

    nc = _build_bass(plans, ntu, nti, debug=debug)

    uf = np.asarray(inputs["user_feat"], np.float32)
    itf = np.asarray(inputs["item_feat"], np.float32)
    proj_u = _proj_host(uf, HALF_U, ntu)
    proj_i = _proj_host(itf, HALF_I, nti)

    f16 = np.float16
    w_pu = np.ascontiguousarray(
        np.asarray(inputs["W_pu"], np.float32).reshape(4, P, D)
        .transpose(1, 0, 2)).astype(f16)
    w_pi = np.ascontiguousarray(
        np.asarray(inputs["W_pi"], np.float32).reshape(4, P, D)
        .transpose(1, 0, 2)).astype(f16)
    b_bc = np.ascontiguousarray(np.broadcast_to(
        np.stack([np.asarray(inputs["b_pu"], np.float32),
                  np.asarray(inputs["b_pi"], np.float32)])[None],
        (P, 2, D))).astype(np.float32)

    def sagew(pre):
        s0 = np.asarray(inputs[f"{pre}_self0"], np.float32)
        a0 = np.asarray(inputs[f"{pre}_agg0"], np.float32) * (1.0 / F0)
        s1 = np.asarray(inputs[f"{pre}_self1"], np.float32)
        a1 = np.asarray(inputs[f"{pre}_agg1"], np.float32) * (1.0 / F0)
        cat = np.concatenate([s0, a0, s1, a1], axis=1)  # [256, 768]
        return cat.reshape(2, P, 768).transpose(1, 0, 2)

    wsage = np.ascontiguousarray(
        np.concatenate([sagew("u"), sagew("i")], axis=2)).astype(f16)
    wlin = np.asarray(inputs["W_lin"], np.float32).astype(f16)
    blin = np.asarray(inputs["b_lin"], np.float32).reshape(1, 1)

    in_maps = []
    for c in range(NCORES):
        par = c % 2
        m = {
            "xt_u": proj_u[par][0], "prow_u": proj_u[par][1],
            "xt_i": proj_i[par][0], "prow_i": proj_i[par][1],
            "w_pu": w_pu, "w_pi": w_pi, "b_bc": b_bc,
            "wsage": wsage, "wlin": wlin, "blin": blin,
        }
        for sd in ("s", "d"):
            p0, p1, seg2 = plans[sd + "0"], plans[sd + "1"], plans[sd + "2"]
            m[f"idx{sd}0"] = _wrap16(p0.idx[c])
            m[f"rid{sd}0"] = _wrap16(p0.rid[c])
            m[f"idx{sd}1"] = _wrap16(p1.idx[c])
            m[f"rid{sd}1"] = _wrap16(p1.rid[c])
            m[f"idx{sd}2"] = np.concatenate(
                [_wrap16(pl.idx[c]) for pl in seg2], axis=1)
            w1, _, _, G1 = plans[sd + "1G"]
            w2, _, _, G2 = plans[sd + "2G"]
            nt1 = p1.tot // P
            nt2 = seg2[0].tot // P
            # G[core][seg, tile, p, w] -> [P, nseg*ntiles*w]
            m[f"g{sd}1"] = np.ascontiguousarray(
                G1[c].transpose(2, 0, 1, 3).reshape(P, nt1 * w1))
            m[f"g{sd}2"] = np.ascontiguousarray(
                G2[c].transpose(2, 0, 1, 3).reshape(P, NSEG * nt2 * w2))
        in_maps.append(m)

    kw = dict(trace=True, trace_cores=list(range(NCORES))) if trace else {}
    res = run_bass_kernel_spmd(nc, in_maps, core_ids=list(range(NCORES)), **kw)
    if trace and res.exec_time_ns:
        print(f"HW exec time: {res.exec_time_ns} ns")
        kernel.last_exec_ns = res.exec_time_ns
    if debug:
        kernel.last_dbg = [res.results[c]["dbg"] for c in range(NCORES)]

    logits = np.concatenate([res.results[c]["out"][0] for c in range(NCORES)])
    return logits.reshape(B, 1).astype(np.float32)
